# revision 4
# baseline (speedup 1.0000x reference)
"""Trainium2 Bass kernel for nn_CapsuleLayer (dynamic routing, 3 iterations).

Math (reference):
    u_hat[b,c,u,s] = sum_i W[c,u,s,i] x[b,i,c]          (B=256,C=1152,U=10,S=16,I=8)
    3x routing:  c_ij = softmax_u(b_ij);  s_j = sum_c c_ij*u_hat;  v = squash(s_j)
                 b_ij += mean_b(u_hat . v)
    return v[..., None]

u_hat is 189MB in fp32 — never materialized.  Both contractions against
u_hat factor through x and W directly:
    s_j[b,us]  = sum_{ci} x[ci,b] * (c_ij[c,u] * W[ci,us])       (PE matmuls)
    a[c,u]     = sum_{s,i} W[ci,us] * G[ci,us],
    G[ci,us]   = sum_b x[b,ci] * v[b,us]                          (PE matmuls)
The W.G elementwise product + s-reduce runs on DVE; the i-reduce (partition
groups of 8) via a tiny selection matmul on PE.  Data-parallel over batch:
each core takes 32 batches; the per-iteration agreement partial [1152,10]
is AllReduce-summed (x1/256 folded into the selection matrix).

K-ordering: k=(c,i), chunk kb holds c in [16kb,16kb+16), partition
p = (c%16)*8 + i.  All heavy tensors are host-pre-transposed so every big
DMA is contiguous.
"""

import numpy as np

import concourse.bass as bass
import concourse.bacc as bacc
import concourse.tile as tile
from concourse import mybir
from concourse import bass_utils

# ---------------------------------------------------------------- constants
B, I, C, U, S = 256, 8, 1152, 10, 16
NCORES = 8
BL = B // NCORES            # 32 batches per core
KT = C // 16                # 72 K-chunks of 128 (16 c x 8 i)
NUS = U * S                 # 160
ROUNDS = 8
CPR = KT // ROUNDS          # 9 chunks per round
EPS = 1e-8
NUM_ROUTING = 3

# dtype config for the two matmul families ("f32" | "f32r" | "bf16")
MM_CFG = "f32"

_DT = {
    "f32": mybir.dt.float32,
    "f32r": mybir.dt.float32,   # storage dtype; bitcast at matmul callsites
    "bf16": mybir.dt.bfloat16,
}
_NP_DT = {
    "f32": np.float32,
    "f32r": np.float32,
    "bf16": None,  # ml_dtypes.bfloat16 resolved lazily
}


def _np_dt(cfg):
    if cfg == "bf16":
        import ml_dtypes
        return ml_dtypes.bfloat16
    return np.float32


def _mm_ap(ap, cfg):
    """Bitcast a matmul operand AP for the f32r PE mode."""
    if cfg == "f32r":
        return ap.bitcast(mybir.dt.float32r)
    return ap


# ---------------------------------------------------------------- device code
def build_nc(cfg=MM_CFG):
    nc = bacc.Bacc(
        "TRN2",
        target_bir_lowering=False,
        debug=False,
        num_devices=NCORES,
    )
    mdt = _DT[cfg]
    f32 = mybir.dt.float32

    w_d = nc.dram_tensor("w_sb", [128, KT * NUS], mdt, kind="ExternalInput")
    # fp32 copy of W for the DVE agreement product when PE runs bf16
    w32_d = (
        nc.dram_tensor("w_f32", [128, KT * NUS], f32, kind="ExternalInput")
        if cfg == "bf16"
        else None
    )
    xt_d = nc.dram_tensor("x_t", [128, KT * BL], mdt, kind="ExternalInput")
    xb_d = nc.dram_tensor("x_b", [BL, KT * 128], mdt, kind="ExternalInput")
    sel_d = nc.dram_tensor("sel", [128, 16], f32, kind="ExternalInput")
    out_d = nc.dram_tensor("v_out", [BL, NUS], f32, kind="ExternalOutput")

    with tile.TileContext(nc) as tc:
        with (
            tc.tile_pool(name="singles", bufs=1) as singles,
            tc.tile_pool(name="weff_p", bufs=3) as weff_p,
            tc.tile_pool(name="prod_p", bufs=3) as prod_p,
            tc.tile_pool(name="rsum_p", bufs=3) as rsum_p,
            tc.tile_pool(name="small", bufs=2) as small,
            tc.tile_pool(name="bsoft", bufs=2) as bsoft,
            tc.tile_pool(name="ps_s", bufs=1, space="PSUM") as ps_s,
            tc.tile_pool(name="ps_g", bufs=2, space="PSUM") as ps_g,
            tc.tile_pool(name="ps_b", bufs=1, space="PSUM") as ps_b,
            tc.tile_pool(name="dram", bufs=2, space="DRAM") as dram,
        ):
            # ---------------- persistent SBUF loads (contiguous DMAs)
            w_sb = singles.tile([128, KT, U, S], mdt)
            for r in range(ROUNDS):
                nc.sync.dma_start(
                    out=w_sb[:, r * CPR:(r + 1) * CPR, :, :],
                    in_=w_d[:, r * CPR * NUS:(r + 1) * CPR * NUS],
                )
            if cfg == "bf16":
                w32_sb = singles.tile([128, KT, U, S], f32)
                for r in range(ROUNDS):
                    nc.sync.dma_start(
                        out=w32_sb[:, r * CPR:(r + 1) * CPR, :, :],
                        in_=w32_d[:, r * CPR * NUS:(r + 1) * CPR * NUS],
                    )
            else:
                w32_sb = w_sb
            x_t = singles.tile([128, KT, BL], mdt)
            nc.sync.dma_start(out=x_t[:], in_=xt_d[:])
            x_b = singles.tile([BL, KT * 128], mdt)
            nc.sync.dma_start(out=x_b[:], in_=xb_d[:])
            sel = singles.tile([128, 16], f32)
            nc.sync.dma_start(out=sel[:], in_=sel_d[:])
            eps_sb = singles.tile([BL, 1], f32)
            nc.vector.memset(eps_sb[:], EPS)

            b_prev = None  # expanded routing logits [128, KT, U]

            def s_pass(it, c_exp):
                """s_raw = X^T.Weff accumulated over all 72 chunks -> psum."""
                s_ps = ps_s.tile([BL, NUS], f32)
                kb = 0
                for r in range(ROUNDS):
                    if c_exp is None:
                        weff = w_sb
                        base = r * CPR
                    else:
                        weff = weff_p.tile([128, CPR, U, S], mdt, tag="weff")
                        nc.vector.tensor_mul(
                            weff[:],
                            w_sb[:, r * CPR:(r + 1) * CPR, :, :],
                            c_exp[:, r * CPR:(r + 1) * CPR, :, None].broadcast_to(
                                [128, CPR, U, S]
                            ),
                        )
                        base = 0
                    for j in range(CPR):
                        nc.tensor.matmul(
                            out=s_ps[:],
                            lhsT=_mm_ap(x_t[:, kb, :], cfg),
                            rhs=_mm_ap(
                                weff[:, base + j, :, :].rearrange(
                                    "p u s -> p (u s)"
                                ),
                                cfg,
                            ),
                            start=(kb == 0),
                            stop=(kb == KT - 1),
                        )
                        kb += 1
                return s_ps

            def squash(s_ps, alpha, out_dt):
                """v = squash(alpha * s_raw); returns v tile [BL, U, S]."""
                s3 = s_ps[:].rearrange("b (u s) -> b u s", u=U)
                s2 = small.tile([BL, U, S], f32, tag="s2")
                nc.scalar.activation(
                    out=s2[:], in_=s3, func=mybir.ActivationFunctionType.Square
                )
                sq = small.tile([BL, U], f32, tag="sq")
                nc.vector.reduce_sum(out=sq[:], in_=s2[:], axis=mybir.AxisListType.X)
                t = small.tile([BL, U], f32, tag="t")
                if alpha != 1.0:
                    nc.vector.tensor_scalar_mul(t[:], sq[:], alpha * alpha)
                else:
                    t = sq
                # rt = sqrt(t + eps) via exp(0.5*ln(t+eps)) (one ACT table set)
                lnt = small.tile([BL, U], f32, tag="lnt")
                nc.scalar.activation(
                    out=lnt[:], in_=t[:],
                    func=mybir.ActivationFunctionType.Ln, bias=eps_sb[:],
                )
                rt = small.tile([BL, U], f32, tag="rt")
                nc.scalar.activation(
                    out=rt[:], in_=lnt[:],
                    func=mybir.ActivationFunctionType.Exp, scale=0.5,
                )
                d = small.tile([BL, U], f32, tag="d")
                nc.vector.tensor_scalar_add(d[:], t[:], 1.0)
                dd = small.tile([BL, U], f32, tag="dd")
                nc.vector.tensor_mul(dd[:], d[:], rt[:])
                g = small.tile([BL, U], f32, tag="g")
                nc.vector.reciprocal(g[:], dd[:])
                af = small.tile([BL, U], f32, tag="af")
                nc.vector.tensor_mul(af[:], t[:], g[:])
                if alpha != 1.0:
                    nc.vector.tensor_scalar_mul(af[:], af[:], alpha)
                v = small.tile([BL, U, S], out_dt, tag="v")
                nc.vector.tensor_mul(
                    v[:], s3, af[:, :, None].broadcast_to([BL, U, S])
                )
                return v

            def a_pass(v):
                """Agreement partial -> AllReduce -> expanded b logits."""
                b_part = small.tile([16, KT * U], f32, tag="b_part")
                for r in range(ROUNDS):
                    g_ps = ps_g.tile([128, 3, 512], f32, tag="g")
                    for j in range(CPR):
                        kb = r * CPR + j
                        nc.tensor.matmul(
                            out=g_ps[:, j // 3, (j % 3) * NUS:(j % 3) * NUS + NUS],
                            lhsT=_mm_ap(x_b[:, kb * 128:(kb + 1) * 128], cfg),
                            rhs=_mm_ap(v[:].rearrange("b u s -> b (u s)"), cfg),
                            start=True,
                            stop=True,
                        )
                    g_view = g_ps[:, :, :3 * NUS].rearrange(
                        "p a (c m) -> p a c m", c=3
                    )
                    prod = prod_p.tile([128, CPR * NUS], f32, tag="prod")
                    nc.vector.tensor_mul(
                        prod[:].rearrange("p (a c m) -> p a c m", a=3, c=3),
                        w32_sb[:, r * CPR:(r + 1) * CPR, :, :].rearrange(
                            "p k u s -> p k (u s)"
                        ).rearrange("p (a c) m -> p a c m", a=3),
                        g_view,
                    )
                    rsum = rsum_p.tile([128, CPR, U], f32, tag="rsum")
                    nc.vector.reduce_sum(
                        out=rsum[:],
                        in_=prod[:].rearrange("p (k u s) -> p k u s", k=CPR, u=U),
                        axis=mybir.AxisListType.X,
                    )
                    b_ps = ps_b.tile([16, CPR * U], f32, tag="b_ps")
                    nc.tensor.matmul(
                        out=b_ps[:],
                        lhsT=sel[:],
                        rhs=rsum[:].rearrange("p k u -> p (k u)"),
                        start=True,
                        stop=True,
                    )
                    nc.scalar.copy(
                        out=b_part[:, r * CPR * U:(r + 1) * CPR * U], in_=b_ps[:]
                    )
                a_in = dram.tile([16, KT * U], f32, tag="a_in")
                a_out = dram.tile([16, KT * U], f32, tag="a_out")
                nc.sync.dma_start(out=a_in[:], in_=b_part[:])
                nc.gpsimd.collective_compute(
                    "AllReduce",
                    mybir.AluOpType.add,
                    replica_groups=[list(range(NCORES))],
                    ins=[a_in[:].opt()],
                    outs=[a_out[:].opt()],
                )
                a_exp = bsoft.tile([128, KT, U], f32, tag="a_exp")
                nc.sync.dma_start(
                    out=a_exp[:],
                    in_=a_out[:, None, :].broadcast_to([16, 8, KT * U]),
                )
                return a_exp

            def softmax(b_exp):
                e = bsoft.tile([128, KT, U], f32, tag="e")
                nc.scalar.activation(
                    out=e[:], in_=b_exp[:], func=mybir.ActivationFunctionType.Exp
                )
                se = bsoft.tile([128, KT], f32, tag="se")
                nc.vector.reduce_sum(out=se[:], in_=e[:], axis=mybir.AxisListType.X)
                re = bsoft.tile([128, KT], f32, tag="re")
                nc.vector.reciprocal(re[:], se[:])
                c_exp = bsoft.tile([128, KT, U], f32, tag="c_exp")
                nc.vector.tensor_mul(
                    c_exp[:], e[:], re[:, :, None].broadcast_to([128, KT, U])
                )
                return c_exp

            # ------------------------------------------------ routing loop
            c_exp = None
            v = None
            for it in range(NUM_ROUTING):
                alpha = 1.0 / U if it == 0 else 1.0
                s_ps = s_pass(it, c_exp)
                last = it == NUM_ROUTING - 1
                v = squash(s_ps, alpha, f32 if last else _DT[cfg])
                if last:
                    break
                a_exp = a_pass(v)
                if b_prev is None:
                    b_exp = a_exp
                else:
                    b_exp = bsoft.tile([128, KT, U], f32, tag="b_exp")
                    nc.vector.tensor_add(b_exp[:], b_prev[:], a_exp[:])
                b_prev = b_exp
                c_exp = softmax(b_exp)

            nc.sync.dma_start(
                out=out_d[:], in_=v[:].rearrange("b u s -> b (u s)")
            )

    nc.compile()
    return nc


# ---------------------------------------------------------------- host prep
def prep_inputs(x, weight, cfg=MM_CFG):
    """Full inputs -> per-core in_maps with kernel-ready layouts."""
    x = np.asarray(x, dtype=np.float32)
    weight = np.asarray(weight, dtype=np.float32)
    npdt = _np_dt(cfg)

    # W: [C,U,S,I] -> [128, KT, U, S] with p = (c%16)*8 + i
    w = (
        weight.reshape(KT, 16, U, S, I)
        .transpose(1, 4, 0, 2, 3)          # [16, I, KT, U, S]
        .reshape(128, KT * U * S)
    )
    w_mm = np.ascontiguousarray(w, dtype=npdt)
    sel = np.zeros((128, 16), np.float32)
    sel[np.arange(128), np.arange(128) // 8] = 1.0 / B

    in_maps = []
    for k in range(NCORES):
        xs = x[k * BL:(k + 1) * BL]                      # [BL, I, C]
        xcib = xs.transpose(2, 1, 0).reshape(KT, 16, I, BL)  # c-major
        x_t = (
            xcib.reshape(KT, 128, BL).transpose(1, 0, 2).reshape(128, KT * BL)
        )
        x_b = xs.transpose(0, 2, 1).reshape(BL, KT * 128)    # [BL, (c,i)]
        m = {
            "w_sb": w_mm,
            "x_t": np.ascontiguousarray(x_t, dtype=npdt),
            "x_b": np.ascontiguousarray(x_b, dtype=npdt),
            "sel": sel,
        }
        if cfg == "bf16":
            m["w_f32"] = np.ascontiguousarray(w, dtype=np.float32)
        in_maps.append(m)
    return in_maps


def assemble_output(results):
    out = np.empty((B, U, S, 1), np.float32)
    for k in range(NCORES):
        out[k * BL:(k + 1) * BL] = (
            results[k]["v_out"].astype(np.float32).reshape(BL, U, S, 1)
        )
    return out


_NC_CACHE = {}


def _get_nc(cfg=MM_CFG):
    if cfg not in _NC_CACHE:
        _NC_CACHE[cfg] = build_nc(cfg)
    return _NC_CACHE[cfg]


def kernel(x, weight):
    nc = _get_nc()
    in_maps = prep_inputs(x, weight)
    res = bass_utils.run_bass_kernel_spmd(
        nc, in_maps, core_ids=list(range(NCORES))
    )
    return assemble_output(res.results)


# revision 6
# speedup vs baseline: 11.3607x; 11.3607x over previous
"""Trainium2 Bass kernel for nn_CapsuleLayer (dynamic routing, 3 iterations).

Math (reference):
    u_hat[b,c,u,s] = sum_i W[c,u,s,i] x[b,i,c]          (B=256,C=1152,U=10,S=16,I=8)
    3x routing:  c_ij = softmax_u(b_ij);  s_j = sum_c c_ij*u_hat;  v = squash(s_j)
                 b_ij += mean_b(u_hat . v)
    return v[..., None]

u_hat is 189MB in fp32 — never materialized.  Both contractions against
u_hat factor through x and W directly:
    s_j[b,us]  = sum_{ci} x[ci,b] * (c_ij[c,u] * W[ci,us])       (PE matmuls)
    a[c,u]     = sum_{s,i} W[ci,us] * G[ci,us],
    G[ci,us]   = sum_b x[b,ci] * v[b,us]                          (PE matmuls)
The W.G elementwise product + s-reduce runs on DVE; the i-reduce (partition
groups of 8) via a tiny selection matmul on PE.  Data-parallel over batch:
each core takes 32 batches; the per-iteration agreement partial [1152,10]
is AllReduce-summed (x1/256 folded into the selection matrix).

K-ordering: k=(c,i), chunk kb holds c in [16kb,16kb+16), partition
p = (c%16)*8 + i.  All heavy tensors are host-pre-transposed so every big
DMA is contiguous.
"""

import numpy as np

import concourse.bass as bass
import concourse.bacc as bacc
import concourse.tile as tile
from concourse import mybir
from concourse import bass_utils

# ---------------------------------------------------------------- constants
B, I, C, U, S = 256, 8, 1152, 10, 16
NCORES = 8
BL = B // NCORES            # 32 batches per core
KT = C // 16                # 72 K-chunks of 128 (16 c x 8 i)
NUS = U * S                 # 160
ROUNDS = 8
CPR = KT // ROUNDS          # 9 chunks per round
EPS = 1e-8
NUM_ROUTING = 3

# dtype config for the two matmul families ("f32" | "f32r" | "bf16")
MM_CFG = "f32"

_DT = {
    "f32": mybir.dt.float32,
    "f32r": mybir.dt.float32,   # storage dtype; bitcast at matmul callsites
    "bf16": mybir.dt.bfloat16,
}
_NP_DT = {
    "f32": np.float32,
    "f32r": np.float32,
    "bf16": None,  # ml_dtypes.bfloat16 resolved lazily
}


def _np_dt(cfg):
    if cfg == "bf16":
        import ml_dtypes
        return ml_dtypes.bfloat16
    return np.float32


def _mm_ap(ap, cfg):
    """Bitcast a matmul operand AP for the f32r PE mode."""
    if cfg == "f32r":
        return ap.bitcast(mybir.dt.float32r)
    return ap


# ---------------------------------------------------------------- device code
def build_nc(cfg=MM_CFG, repeat=1):
    nc = bacc.Bacc(
        "TRN2",
        target_bir_lowering=False,
        debug=False,
        num_devices=NCORES,
    )
    mdt = _DT[cfg]
    f32 = mybir.dt.float32

    w_d = nc.dram_tensor("w_sb", [128, KT * NUS], mdt, kind="ExternalInput")
    # fp32 copy of W for the DVE agreement product when PE runs bf16
    w32_d = (
        nc.dram_tensor("w_f32", [128, KT * NUS], f32, kind="ExternalInput")
        if cfg == "bf16"
        else None
    )
    xt_d = nc.dram_tensor("x_t", [128, KT * BL], mdt, kind="ExternalInput")
    xb_d = nc.dram_tensor("x_b", [BL, KT * 128], mdt, kind="ExternalInput")
    sel_d = nc.dram_tensor("sel", [128, 16], f32, kind="ExternalInput")
    out_d = nc.dram_tensor("v_out", [BL, NUS], f32, kind="ExternalOutput")

    with tile.TileContext(nc) as tc:
        with (
            tc.tile_pool(name="singles", bufs=1) as singles,
            tc.tile_pool(name="weff_p", bufs=3) as weff_p,
            tc.tile_pool(name="prod_p", bufs=3) as prod_p,
            tc.tile_pool(name="rsum_p", bufs=3) as rsum_p,
            tc.tile_pool(name="small", bufs=2) as small,
            tc.tile_pool(name="bsoft", bufs=2) as bsoft,
            tc.tile_pool(name="ps_s", bufs=1, space="PSUM") as ps_s,
            tc.tile_pool(name="ps_g", bufs=2, space="PSUM") as ps_g,
            tc.tile_pool(name="ps_b", bufs=1, space="PSUM") as ps_b,
            tc.tile_pool(name="dram", bufs=2, space="DRAM") as dram,
        ):
            # ---------------- persistent SBUF loads (contiguous DMAs)
            w_sb = singles.tile([128, KT, U, S], mdt)
            for r in range(ROUNDS):
                nc.sync.dma_start(
                    out=w_sb[:, r * CPR:(r + 1) * CPR, :, :],
                    in_=w_d[:, r * CPR * NUS:(r + 1) * CPR * NUS],
                )
            if cfg == "bf16":
                w32_sb = singles.tile([128, KT, U, S], f32)
                for r in range(ROUNDS):
                    nc.sync.dma_start(
                        out=w32_sb[:, r * CPR:(r + 1) * CPR, :, :],
                        in_=w32_d[:, r * CPR * NUS:(r + 1) * CPR * NUS],
                    )
            else:
                w32_sb = w_sb
            x_t = singles.tile([128, KT, BL], mdt)
            nc.sync.dma_start(out=x_t[:], in_=xt_d[:])
            x_b = singles.tile([BL, KT * 128], mdt)
            nc.sync.dma_start(out=x_b[:], in_=xb_d[:])
            sel = singles.tile([128, 16], f32)
            nc.sync.dma_start(out=sel[:], in_=sel_d[:])
            eps_sb = singles.tile([BL, 1], f32)
            nc.vector.memset(eps_sb[:], EPS)

            b_prev = None  # expanded routing logits [128, KT, U]

            def s_pass(it, c_exp):
                """s_raw = X^T.Weff accumulated over all 72 chunks -> psum."""
                s_ps = ps_s.tile([BL, NUS], f32)
                kb = 0
                for r in range(ROUNDS):
                    if c_exp is None:
                        weff = w_sb
                        base = r * CPR
                    else:
                        weff = weff_p.tile([128, CPR, U, S], mdt, tag="weff")
                        nc.vector.tensor_mul(
                            weff[:],
                            w_sb[:, r * CPR:(r + 1) * CPR, :, :],
                            c_exp[:, r * CPR:(r + 1) * CPR, :, None].broadcast_to(
                                [128, CPR, U, S]
                            ),
                        )
                        base = 0
                    for j in range(CPR):
                        nc.tensor.matmul(
                            out=s_ps[:],
                            lhsT=_mm_ap(x_t[:, kb, :], cfg),
                            rhs=_mm_ap(
                                weff[:, base + j, :, :].rearrange(
                                    "p u s -> p (u s)"
                                ),
                                cfg,
                            ),
                            start=(kb == 0),
                            stop=(kb == KT - 1),
                        )
                        kb += 1
                return s_ps

            def squash(s_ps, alpha, out_dt):
                """v = squash(alpha * s_raw); returns v tile [BL, U, S]."""
                s3 = s_ps[:].rearrange("b (u s) -> b u s", u=U)
                s2 = small.tile([BL, U, S], f32, tag="s2")
                nc.scalar.activation(
                    out=s2[:], in_=s3, func=mybir.ActivationFunctionType.Square
                )
                sq = small.tile([BL, U], f32, tag="sq")
                nc.vector.reduce_sum(out=sq[:], in_=s2[:], axis=mybir.AxisListType.X)
                t = small.tile([BL, U], f32, tag="t")
                if alpha != 1.0:
                    nc.vector.tensor_scalar_mul(t[:], sq[:], alpha * alpha)
                else:
                    t = sq
                # rt = sqrt(t + eps) via exp(0.5*ln(t+eps)) (one ACT table set)
                lnt = small.tile([BL, U], f32, tag="lnt")
                nc.scalar.activation(
                    out=lnt[:], in_=t[:],
                    func=mybir.ActivationFunctionType.Ln, bias=eps_sb[:],
                )
                rt = small.tile([BL, U], f32, tag="rt")
                nc.scalar.activation(
                    out=rt[:], in_=lnt[:],
                    func=mybir.ActivationFunctionType.Exp, scale=0.5,
                )
                d = small.tile([BL, U], f32, tag="d")
                nc.vector.tensor_scalar_add(d[:], t[:], 1.0)
                dd = small.tile([BL, U], f32, tag="dd")
                nc.vector.tensor_mul(dd[:], d[:], rt[:])
                g = small.tile([BL, U], f32, tag="g")
                nc.vector.reciprocal(g[:], dd[:])
                af = small.tile([BL, U], f32, tag="af")
                nc.vector.tensor_mul(af[:], t[:], g[:])
                if alpha != 1.0:
                    nc.vector.tensor_scalar_mul(af[:], af[:], alpha)
                v = small.tile([BL, U, S], out_dt, tag="v")
                nc.vector.tensor_mul(
                    v[:], s3, af[:, :, None].broadcast_to([BL, U, S])
                )
                return v

            def a_pass(v):
                """Agreement partial -> AllReduce -> expanded b logits."""
                b_part = small.tile([16, KT * U], f32, tag="b_part")
                for r in range(ROUNDS):
                    g_ps = ps_g.tile([128, 3, 512], f32, tag="g")
                    for j in range(CPR):
                        kb = r * CPR + j
                        nc.tensor.matmul(
                            out=g_ps[:, j // 3, (j % 3) * NUS:(j % 3) * NUS + NUS],
                            lhsT=_mm_ap(x_b[:, kb * 128:(kb + 1) * 128], cfg),
                            rhs=_mm_ap(v[:].rearrange("b u s -> b (u s)"), cfg),
                            start=True,
                            stop=True,
                        )
                    g_view = g_ps[:, :, :3 * NUS].rearrange(
                        "p a (c m) -> p a c m", c=3
                    )
                    prod = prod_p.tile([128, CPR * NUS], f32, tag="prod")
                    nc.vector.tensor_mul(
                        prod[:].rearrange("p (a c m) -> p a c m", a=3, c=3),
                        w32_sb[:, r * CPR:(r + 1) * CPR, :, :].rearrange(
                            "p k u s -> p k (u s)"
                        ).rearrange("p (a c) m -> p a c m", a=3),
                        g_view,
                    )
                    rsum = rsum_p.tile([128, CPR, U], f32, tag="rsum")
                    nc.vector.reduce_sum(
                        out=rsum[:],
                        in_=prod[:].rearrange("p (k u s) -> p k u s", k=CPR, u=U),
                        axis=mybir.AxisListType.X,
                    )
                    b_ps = ps_b.tile([16, CPR * U], f32, tag="b_ps")
                    nc.tensor.matmul(
                        out=b_ps[:],
                        lhsT=sel[:],
                        rhs=rsum[:].rearrange("p k u -> p (k u)"),
                        start=True,
                        stop=True,
                    )
                    nc.scalar.copy(
                        out=b_part[:, r * CPR * U:(r + 1) * CPR * U], in_=b_ps[:]
                    )
                a_in = dram.tile([16, KT * U], f32, tag="a_in")
                a_out = dram.tile([16, KT * U], f32, tag="a_out")
                nc.sync.dma_start(out=a_in[:], in_=b_part[:])
                nc.gpsimd.collective_compute(
                    "AllReduce",
                    mybir.AluOpType.add,
                    replica_groups=[list(range(NCORES))],
                    ins=[a_in[:].opt()],
                    outs=[a_out[:].opt()],
                )
                a_exp = bsoft.tile([128, KT, U], f32, tag="a_exp")
                nc.sync.dma_start(
                    out=a_exp[:],
                    in_=a_out[:, None, :].broadcast_to([16, 8, KT * U]),
                )
                return a_exp

            def softmax(b_exp):
                e = bsoft.tile([128, KT, U], f32, tag="e")
                nc.scalar.activation(
                    out=e[:], in_=b_exp[:], func=mybir.ActivationFunctionType.Exp
                )
                se = bsoft.tile([128, KT], f32, tag="se")
                nc.vector.reduce_sum(out=se[:], in_=e[:], axis=mybir.AxisListType.X)
                re = bsoft.tile([128, KT], f32, tag="re")
                nc.vector.reciprocal(re[:], se[:])
                c_exp = bsoft.tile([128, KT, U], f32, tag="c_exp")
                nc.vector.tensor_mul(
                    c_exp[:], e[:], re[:, :, None].broadcast_to([128, KT, U])
                )
                return c_exp

            # ------------------------------------------------ routing loop
            for _rep in range(repeat):
                b_prev = None
                c_exp = None
                v = None
                for it in range(NUM_ROUTING):
                    alpha = 1.0 / U if it == 0 else 1.0
                    s_ps = s_pass(it, c_exp)
                    last = it == NUM_ROUTING - 1
                    v = squash(s_ps, alpha, f32 if last else _DT[cfg])
                    if last:
                        break
                    a_exp = a_pass(v)
                    if b_prev is None:
                        b_exp = a_exp
                    else:
                        b_exp = bsoft.tile([128, KT, U], f32, tag="b_exp")
                        nc.vector.tensor_add(b_exp[:], b_prev[:], a_exp[:])
                    b_prev = b_exp
                    c_exp = softmax(b_exp)

                nc.sync.dma_start(
                    out=out_d[:], in_=v[:].rearrange("b u s -> b (u s)")
                )

    nc.compile()
    return nc


# ---------------------------------------------------------------- host prep
def prep_inputs(x, weight, cfg=MM_CFG):
    """Full inputs -> per-core in_maps with kernel-ready layouts."""
    x = np.asarray(x, dtype=np.float32)
    weight = np.asarray(weight, dtype=np.float32)
    npdt = _np_dt(cfg)

    # W: [C,U,S,I] -> [128, KT, U, S] with p = (c%16)*8 + i
    w = (
        weight.reshape(KT, 16, U, S, I)
        .transpose(1, 4, 0, 2, 3)          # [16, I, KT, U, S]
        .reshape(128, KT * U * S)
    )
    w_mm = np.ascontiguousarray(w, dtype=npdt)
    sel = np.zeros((128, 16), np.float32)
    sel[np.arange(128), np.arange(128) // 8] = 1.0 / B

    in_maps = []
    for k in range(NCORES):
        xs = x[k * BL:(k + 1) * BL]                      # [BL, I, C]
        xcib = xs.transpose(2, 1, 0).reshape(KT, 16, I, BL)  # c-major
        x_t = (
            xcib.reshape(KT, 128, BL).transpose(1, 0, 2).reshape(128, KT * BL)
        )
        x_b = xs.transpose(0, 2, 1).reshape(BL, KT * 128)    # [BL, (c,i)]
        m = {
            "w_sb": w_mm,
            "x_t": np.ascontiguousarray(x_t, dtype=npdt),
            "x_b": np.ascontiguousarray(x_b, dtype=npdt),
            "sel": sel,
        }
        if cfg == "bf16":
            m["w_f32"] = np.ascontiguousarray(w, dtype=np.float32)
        in_maps.append(m)
    return in_maps


def assemble_output(results):
    out = np.empty((B, U, S, 1), np.float32)
    for k in range(NCORES):
        out[k * BL:(k + 1) * BL] = (
            results[k]["v_out"].astype(np.float32).reshape(BL, U, S, 1)
        )
    return out


_NC_CACHE = {}


def _get_nc(cfg=MM_CFG):
    if cfg not in _NC_CACHE:
        _NC_CACHE[cfg] = build_nc(cfg)
    return _NC_CACHE[cfg]


def kernel(x, weight):
    nc = _get_nc()
    in_maps = prep_inputs(x, weight)
    res = bass_utils.run_bass_kernel_spmd(
        nc, in_maps, core_ids=list(range(NCORES))
    )
    return assemble_output(res.results)
